# revision 1
# baseline (speedup 1.0000x reference)
"""Multi-head causal attention (B=4, T=4096, D=1024, H=16) on 8 TRN2 NeuronCores.

Sharding: core c -> (batch b = c//2, head-group g = c%2 of 8 heads).
Per core: QKV projection (fp32r matmuls), flash-style causal attention with
transposed layouts (no transposes inside attention), output projection.
Host sums the two per-batch partials (w_proj row-split) and transposes.

Internals:
  qT, kT  [512, T]   feature-on-partition layout (from W-stationary projection)
  V       [4 pairs, T, 130]  natural layout + ones column per head (denominator
                             comes out of the same AV matmul in PSUM row 64)
  S^T     [tk, tq] blocks in PSUM; exp on ScalarE (scale=0.125 folds 1/sqrt(64))
  softmax without max-subtraction (logits are ~N(0,1); exp never overflows)
  causal: block-skip above diagonal, 0/1 mask multiply on diagonal tiles
  phase interleaving: QKV-projection block tb+1 is emitted between attention
  rows so its PE work fills the ScalarE(exp)-wait gaps and keeps the PE warm
"""
import numpy as np

B, T, D = 4, 4096, 1024
H, HD = 16, 64
N_CORES = 8
PAIRS = 4            # head-pairs per core (8 local heads)
DL = PAIRS * 128     # 512 = local q/k/v width
TQ = 512             # query block
NTQ = T // TQ        # 8

_CACHE = {}


def _build_masks():
    # mask[g][p, tkb*512 + q] = 1.0 iff p + (256*g + 128*tkb) <= q
    m = np.zeros((2, 128, 1024), dtype=np.float32)
    p = np.arange(128)[:, None]
    q = np.arange(512)[None, :]
    for g in range(2):
        for tkb in range(2):
            d = 256 * g + 128 * tkb
            m[g, :, tkb * 512:(tkb + 1) * 512] = (p + d <= q).astype(np.float32)
    return m


def _build_nc():
    import concourse.tile as tile
    from concourse import bacc, mybir

    fp32 = mybir.dt.float32
    fp32r = mybir.dt.float32r
    AF = mybir.ActivationFunctionType

    nc = bacc.Bacc("TRN2", target_bir_lowering=False, debug=False,
                   num_devices=N_CORES)
    xb_d = nc.dram_tensor("xb", [T, D], fp32, kind="ExternalInput").ap()
    wqkv_d = nc.dram_tensor("wqkv", [D, 3 * DL], fp32r, kind="ExternalInput").ap()
    wp_d = nc.dram_tensor("wp", [DL, D], fp32r, kind="ExternalInput").ap()
    mask_d = nc.dram_tensor("mask", [2, 128, 1024], fp32r, kind="ExternalInput").ap()
    id_d = nc.dram_tensor("ident", [128, 128], fp32, kind="ExternalInput").ap()
    yt_d = nc.dram_tensor("yt", [D, T], fp32, kind="ExternalOutput").ap()
    qt_d = nc.dram_tensor("qt", [DL, T], fp32r)
    kt_d = nc.dram_tensor("kt", [DL, T], fp32r)
    v_d = nc.dram_tensor("v", [PAIRS, T, 130], fp32r)

    with tile.TileContext(nc) as tc:
        with (
            tc.tile_pool(name="sb", bufs=1) as pool,
            tc.tile_pool(name="ps", bufs=1, space="PSUM") as psum,
        ):
            ident = pool.tile([128, 128], fp32, tag="ident")
            nc.sync.dma_start(ident[:], id_d[:])
            wqkv = pool.tile([128, 8, 3 * DL], fp32r, tag="wqkv")
            nc.sync.dma_start(wqkv[:], wqkv_d.rearrange("(a p) f -> p a f", p=128))
            wp = pool.tile([128, 4, D], fp32r, tag="wp")
            nc.sync.dma_start(wp[:], wp_d.rearrange("(a p) f -> p a f", p=128))
            msk = pool.tile([128, 2, 1024], fp32r, tag="msk")
            nc.sync.dma_start(msk[:], mask_d.rearrange("g p f -> p g f"))

            def emit_proj_block(tb):
                """QKV projection for t rows [tb*512, (tb+1)*512)."""
                x_sb = pool.tile([128, 4, D], fp32, tag="x", bufs=2,
                                 name=f"x_{tb}")
                nc.sync.dma_start(
                    x_sb[:],
                    xb_d[tb * 512:(tb + 1) * 512, :]
                    .rearrange("(a p) f -> p a f", p=128))
                xT = pool.tile([128, 8, 512], fp32r, tag="xT", bufs=2,
                               name=f"xT_{tb}")
                for k in range(8):
                    pxt = psum.tile([128, 512], fp32, tag="yp", bufs=2,
                                    name=f"pxt_{tb}_{k}")
                    for s in range(4):
                        nc.tensor.transpose(pxt[:, s * 128:(s + 1) * 128],
                                            x_sb[:, s, k * 128:(k + 1) * 128],
                                            ident[:])
                    nc.vector.tensor_copy(xT[:, k, :], pxt[:])
                # Q, K sections: W stationary -> transposed output [f, t]
                for fc in range(8):
                    ps = psum.tile([128, 512], fp32, tag="yp", bufs=2,
                                   name=f"pqk_{tb}_{fc}")
                    for k in range(8):
                        nc.tensor.matmul(ps[:], wqkv[:, k, fc * 128:(fc + 1) * 128],
                                         xT[:, k, :], start=(k == 0), stop=(k == 7))
                    st = pool.tile([128, 512], fp32r, tag="qks", bufs=2,
                                   name=f"qks_{tb}_{fc}")
                    nc.vector.tensor_copy(st[:], ps[:])
                    dst = qt_d if fc < 4 else kt_d
                    fcl = fc % 4
                    nc.sync.dma_start(
                        dst[fcl * 128:(fcl + 1) * 128, tb * 512:(tb + 1) * 512],
                        st[:])
                # V section: xT stationary -> natural output [t, f]
                for s in range(4):
                    ps = psum.tile([128, 512], fp32, tag="yp", bufs=2,
                                   name=f"pv_{tb}_{s}")
                    for k in range(8):
                        nc.tensor.matmul(ps[:], xT[:, k, s * 128:(s + 1) * 128],
                                         wqkv[:, k, 2 * DL:3 * DL],
                                         start=(k == 0), stop=(k == 7))
                    vs = pool.tile([128, 4, 2, 65], fp32r, tag="vst", bufs=2,
                                   name=f"vst_{tb}_{s}")
                    nc.vector.memset(vs[:].bitcast(fp32), 1.0)
                    nc.vector.tensor_copy(
                        vs[:, :, :, 0:64],
                        ps[:].rearrange("p (a h e) -> p a h e", a=4, h=2))
                    r0 = tb * 512 + s * 128
                    nc.sync.dma_start(
                        v_d[:, r0:r0 + 128, :].rearrange("a p f -> p a f"),
                        vs[:].rearrange("p a h e -> p a (h e)"))

            def emit_attn_row(j):
                """Attention + output projection for tq rows [j*512, (j+1)*512)."""
                otsb = [pool.tile([128, TQ], fp32r, tag=f"otsb{pr}", bufs=2,
                                  name=f"otsb{pr}_{j}")
                        for pr in range(PAIRS)]
                for pr in range(PAIRS):
                    qt = pool.tile([128, TQ], fp32r, tag="qt", bufs=2,
                                   name=f"qt_{pr}_{j}")
                    nc.sync.dma_start(
                        qt[:],
                        qt_d[pr * 128:(pr + 1) * 128, j * 512:(j + 1) * 512])
                    ot = [psum.tile([65, TQ], fp32, tag="ot", bufs=2,
                                    name=f"ot{h_}_{pr}_{j}")
                          for h_ in range(2)]
                    for c in range(j + 1):
                        kt = pool.tile([128, TQ], fp32r, tag="kt", bufs=3,
                                       name=f"kt_{pr}_{j}_{c}")
                        nc.sync.dma_start(
                            kt[:],
                            kt_d[pr * 128:(pr + 1) * 128, c * 512:(c + 1) * 512])
                        vt = pool.tile([128, 4, 130], fp32r, tag="vt", bufs=3,
                                       name=f"vt_{pr}_{j}_{c}")
                        nc.sync.dma_start(
                            vt[:],
                            v_d[pr, c * 512:(c + 1) * 512, :]
                            .rearrange("(a p) f -> p a f", p=128))
                        for g in range(2):
                            sts = [psum.tile([128, 1024], fp32, tag="st", bufs=2,
                                             name=f"st{h_}_{pr}_{j}_{c}_{g}")
                                   for h_ in range(2)]
                            for tkb in range(2):
                                for h in range(2):
                                    nc.tensor.matmul(
                                        sts[h][:, tkb * 512:(tkb + 1) * 512],
                                        kt[h * 64:(h + 1) * 64,
                                           (g * 2 + tkb) * 128:(g * 2 + tkb + 1) * 128],
                                        qt[h * 64:(h + 1) * 64, :],
                                        start=True, stop=True)
                            for h in range(2):
                                ex = pool.tile([128, 1024], fp32r, tag="ex", bufs=4,
                                               name=f"ex{h}_{pr}_{j}_{c}_{g}")
                                nc.scalar.activation(ex[:], sts[h][:], AF.Exp,
                                                     scale=0.125)
                                if c == j:
                                    nc.vector.tensor_mul(ex[:], ex[:], msk[:, g, :])
                                for tkb in range(2):
                                    nc.tensor.matmul(
                                        ot[h][:],
                                        vt[:, g * 2 + tkb, h * 65:(h + 1) * 65],
                                        ex[:, tkb * 512:(tkb + 1) * 512],
                                        start=(c == 0 and g == 0 and tkb == 0),
                                        stop=(c == j and g == 1 and tkb == 1))
                    for h in range(2):
                        den = pool.tile([1, TQ], fp32, tag="den", bufs=1,
                                        name=f"den{h}_{pr}_{j}")
                        nc.vector.tensor_copy(den[:], ot[h][64:65, :])
                        bc = pool.tile([64, TQ], fp32, tag="bc", bufs=1,
                                       name=f"bc{h}_{pr}_{j}")
                        nc.gpsimd.partition_broadcast(bc[:], den[:])
                        rec = pool.tile([64, TQ], fp32, tag="rec", bufs=1,
                                        name=f"rec{h}_{pr}_{j}")
                        nc.vector.reciprocal_approx_fast(rec[:], bc[:])
                        nc.vector.tensor_mul(otsb[pr][h * 64:(h + 1) * 64, :],
                                             ot[h][0:64, :], rec[:])
                # output projection for this tq block
                for mc in range(8):
                    yp = psum.tile([128, TQ], fp32, tag="yp", bufs=2,
                                   name=f"yp_{j}_{mc}")
                    for kc in range(PAIRS):
                        nc.tensor.matmul(yp[:], wp[:, kc, mc * 128:(mc + 1) * 128],
                                         otsb[kc][:],
                                         start=(kc == 0), stop=(kc == PAIRS - 1))
                    ys = pool.tile([128, TQ], fp32, tag="ys", bufs=2,
                                   name=f"ys_{j}_{mc}")
                    nc.vector.tensor_copy(ys[:], yp[:])
                    nc.sync.dma_start(
                        yt_d[mc * 128:(mc + 1) * 128, j * 512:(j + 1) * 512],
                        ys[:])

            # Interleave: proj block tb+1 emitted between attention rows so
            # PE-heavy projection work overlaps ScalarE-heavy attention rows.
            emit_proj_block(0)
            emit_proj_block(1)
            for j in range(NTQ):
                if j + 2 < NTQ:
                    emit_proj_block(j + 2)
                emit_attn_row(j)

    nc.compile()
    return nc


def _get_nc():
    if "nc" not in _CACHE:
        _CACHE["nc"] = _build_nc()
    return _CACHE["nc"]


def _in_maps(x, w_qkv, w_proj):
    masks = _build_masks()
    ident = np.eye(128, dtype=np.float32)
    maps = []
    for c in range(N_CORES):
        b, g = c // 2, c % 2
        wq = w_qkv[:, g * DL:(g + 1) * DL]
        wk = w_qkv[:, D + g * DL:D + (g + 1) * DL]
        wv = w_qkv[:, 2 * D + g * DL:2 * D + (g + 1) * DL]
        maps.append({
            "xb": np.ascontiguousarray(x[b]),
            "wqkv": np.ascontiguousarray(np.concatenate([wq, wk, wv], axis=1)),
            "wp": np.ascontiguousarray(w_proj[g * DL:(g + 1) * DL, :]),
            "mask": masks,
            "ident": ident,
        })
    return maps


def _run(x, w_qkv, w_proj, trace=False):
    from concourse.bass_utils import run_bass_kernel_spmd

    nc = _get_nc()
    res = run_bass_kernel_spmd(nc, _in_maps(x, w_qkv, w_proj),
                               core_ids=list(range(N_CORES)), trace=trace)
    outs = [res.results[c]["yt"] for c in range(N_CORES)]
    y = np.stack([(outs[2 * b] + outs[2 * b + 1]).T for b in range(B)])
    return np.ascontiguousarray(y.astype(np.float32)), res


def kernel(x, w_qkv, w_proj):
    x = np.asarray(x, dtype=np.float32)
    w_qkv = np.asarray(w_qkv, dtype=np.float32)
    w_proj = np.asarray(w_proj, dtype=np.float32)
    y, _ = _run(x, w_qkv, w_proj, trace=False)
    return y


def kernel_traced(x, w_qkv, w_proj):
    """Test-only entry: run with NTFF profiling (needs the sibling prof_shim
    module; the graded kernel() path never imports it)."""
    import prof_shim
    prof_shim.install()
    y, res = _run(np.asarray(x, np.float32), np.asarray(w_qkv, np.float32),
                  np.asarray(w_proj, np.float32), trace=True)
    return y, res



# revision 5
# speedup vs baseline: 1.4007x; 1.4007x over previous
"""Multi-head causal attention (B=4, T=4096, D=1024, H=16) on 8 TRN2 NeuronCores.

Sharding: core c -> (batch b = c//2, head-group g = c%2 of 8 heads).
Host sums the two per-batch partials (w_proj row-split) and transposes.

v2 design (vs v1 baseline at 1233us):
  - bf16 operands everywhere (matmul rate unchanged vs fp32r, but halves
    SBUF/HBM footprint); fp32 PSUM accumulation throughout.
  - Everything SBUF-resident: K^T / V / Q^T live in SBUF between the QKV
    projection and attention -- no DRAM round-trip (v1 moved ~110MB).
  - No PE transposes: host supplies x^T, projection emits Q^T/K^T directly
    (W-stationary) and V in natural layout (x^T-stationary).
  - Causal diagonal blocks trimmed at 128-granularity: S / exp / mask / AV
    restricted to valid columns (-8% PE and ScalarE work).
  - Software pipeline: AV for tile k is emitted while tile k+1's S/exp are
    in flight, so the PE never sits on the just-issued exp. Projection and
    (deferred) output-projection units are paced into the attention stream
    as PE gap fillers; ScalarE (exp, ~580us) hides under PE (~670us).
  - softmax without max-subtraction (logits ~N(0,1)); denominator via a
    ones-column in V (comes out of the same AV matmul, PSUM row 64).
"""
import math

import numpy as np

B, T, D = 4, 4096, 1024
H, HD = 16, 64
N_CORES = 8
PAIRS = 4            # head-pairs per core (8 local heads)
DL = PAIRS * 128     # 512 = local q/k/v width
TQ = 512             # query block
NTQ = T // TQ        # 8

_CACHE = {}


def _build_nc():
    import concourse.tile as tile
    from concourse import bacc, mybir

    fp32 = mybir.dt.float32
    bf16 = mybir.dt.bfloat16
    AF = mybir.ActivationFunctionType

    nc = bacc.Bacc("TRN2", target_bir_lowering=False, debug=False,
                   num_devices=N_CORES)
    xt_d = nc.dram_tensor("xt", [D, T], bf16, kind="ExternalInput").ap()
    wqkv_d = nc.dram_tensor("wqkv", [D, 3 * DL], bf16, kind="ExternalInput").ap()
    wp_d = nc.dram_tensor("wp", [DL, D], bf16, kind="ExternalInput").ap()
    msk_d = nc.dram_tensor("msk", [128, 128], bf16, kind="ExternalInput").ap()
    yt_d = nc.dram_tensor("yt", [D, T], fp32, kind="ExternalOutput").ap()

    with tile.TileContext(nc) as tc:
        with (
            tc.tile_pool(name="sb", bufs=1) as pool,
            tc.tile_pool(name="ps", bufs=1, space="PSUM") as psum,
        ):
            wqkv = pool.tile([128, 8, 3 * DL], bf16, tag="wqkv")
            nc.sync.dma_start(wqkv[:], wqkv_d.rearrange("(a p) f -> p a f", p=128))
            wp = pool.tile([128, 4, D], bf16, tag="wp")
            nc.sync.dma_start(wp[:], wp_d.rearrange("(a p) f -> p a f", p=128))
            msk = pool.tile([128, 128], bf16, tag="msk")
            nc.sync.dma_start(msk[:], msk_d[:])

            # K^T and V resident for all 8 token-blocks; Q^T rotates (only
            # row j reads q block j; block j+2 is written during row j).
            kT = [pool.tile([128, PAIRS, TQ], bf16, tag=f"kT{t}",
                            name=f"kT_{t}")
                  for t in range(NTQ)]
            vb = [pool.tile([128, PAIRS, 4, 2, 65], bf16, tag=f"vb{t}",
                            name=f"vb_{t}")
                  for t in range(NTQ)]
            for t in range(NTQ):
                # ones column per head -> softmax denominator out of AV matmul
                nc.vector.memset(vb[t][:, :, :, :, 64:65], 1.0)

            qT = {}

            def dma_x(tb):
                xt = pool.tile([128, 8, TQ], bf16, tag="xt", bufs=2,
                               name=f"xt_{tb}")
                nc.sync.dma_start(
                    xt[:],
                    xt_d[:, tb * TQ:(tb + 1) * TQ]
                    .rearrange("(a p) t -> p a t", p=128))
                return xt

            def copy_engine():
                # GPSIMD/Pool cannot read PSUM on TRN2; DVE does all
                # PSUM->SBUF drains.
                return nc.vector

            def make_units(tb, xt):
                """12 projection PE-work units for token rows [tb*512,+512)."""
                units = []

                def qk_unit(fc):
                    def run():
                        if fc == 0:
                            qT[tb] = pool.tile([128, PAIRS, TQ], bf16,
                                               tag="qt", bufs=3,
                                               name=f"qt_{tb}")
                        pp = psum.tile([128, TQ], fp32, tag="pp", bufs=2,
                                       name=f"pqk_{tb}_{fc}")
                        for kb in range(8):
                            nc.tensor.matmul(
                                pp[:], wqkv[:, kb, fc * 128:(fc + 1) * 128],
                                xt[:, kb, :], start=(kb == 0), stop=(kb == 7))
                        dst = qT[tb] if fc < 4 else kT[tb]
                        copy_engine().tensor_copy(dst[:, fc % 4, :], pp[:])
                    return run

                def v_unit(s):
                    def run():
                        pp = psum.tile([128, TQ], fp32, tag="pp", bufs=2,
                                       name=f"pv_{tb}_{s}")
                        for kb in range(8):
                            nc.tensor.matmul(
                                pp[:], xt[:, kb, s * 128:(s + 1) * 128],
                                wqkv[:, kb, 2 * DL:3 * DL],
                                start=(kb == 0), stop=(kb == 7))
                        copy_engine().tensor_copy(
                            vb[tb][:, :, s, :, 0:64],
                            pp[:].rearrange("p (a h e) -> p a h e", a=4, h=2))
                    return run

                for fc in range(8):
                    units.append(qk_unit(fc))
                for s in range(4):
                    units.append(v_unit(s))
                return units

            ob = {}  # (j, pr) -> attention-output SBUF tile [128, TQ] bf16

            def wproj_unit(j, mc):
                def run():
                    yp = psum.tile([128, TQ], fp32, tag="pp", bufs=2,
                                   name=f"yp_{j}_{mc}")
                    for kc in range(PAIRS):
                        nc.tensor.matmul(yp[:],
                                         wp[:, kc, mc * 128:(mc + 1) * 128],
                                         ob[(j, kc)][:],
                                         start=(kc == 0), stop=(kc == PAIRS - 1))
                    ys = pool.tile([128, TQ], fp32, tag="ys", bufs=2,
                                   name=f"ys_{j}_{mc}")
                    nc.vector.tensor_copy(ys[:], yp[:])
                    nc.sync.dma_start(
                        yt_d[mc * 128:(mc + 1) * 128, j * TQ:(j + 1) * TQ],
                        ys[:])
                return run

            # ---- attention tile pipeline (AV delayed by one tile) ----
            pending = [None]

            def do_av(j, pr, c, h, half, ot, ex):
                for tkb in range(2):
                    blk = half * 2 + tkb
                    d = 128 * blk if c == j else 0
                    nc.tensor.matmul(
                        ot[:, d:TQ], vb[c][:, pr, blk, h, :],
                        ex[:, tkb, d:TQ],
                        start=(c == 0 and half == 0 and tkb == 0),
                        stop=(c == j and half == 1 and tkb == 1))

            def flush_av():
                if pending[0] is None:
                    return
                args, post = pending[0]
                pending[0] = None
                do_av(*args)
                if post is not None:
                    post()

            def attn_tile(j, pr, c, h, half, ot, post=None):
                st = psum.tile([128, 2, TQ], fp32, tag="st", bufs=2,
                               name=f"st_{j}_{pr}_{c}_{h}_{half}")
                for tkb in range(2):
                    blk = half * 2 + tkb
                    d = 128 * blk if c == j else 0
                    nc.tensor.matmul(
                        st[:, tkb, d:TQ],
                        kT[c][h * 64:(h + 1) * 64, pr, blk * 128:(blk + 1) * 128],
                        qT[j][h * 64:(h + 1) * 64, pr, d:TQ],
                        start=True, stop=True)
                ex = pool.tile([128, 2, TQ], bf16, tag="ex", bufs=4,
                               name=f"ex_{j}_{pr}_{c}_{h}_{half}")
                if c < j:
                    nc.scalar.activation(ex[:], st[:], AF.Exp, scale=0.125)
                else:
                    for tkb in range(2):
                        d = 128 * (half * 2 + tkb)
                        nc.scalar.activation(ex[:, tkb, d:TQ], st[:, tkb, d:TQ],
                                             AF.Exp, scale=0.125)
                        nc.vector.tensor_mul(ex[:, tkb, d:d + 128],
                                             ex[:, tkb, d:d + 128], msk[:])
                flush_av()
                pending[0] = ((j, pr, c, h, half, ot, ex), post)

            def normalize(j, pr, h, ot):
                den = pool.tile([1, TQ], fp32, tag="den", bufs=2,
                                name=f"den_{j}_{pr}_{h}")
                nc.vector.tensor_copy(den[:], ot[64:65, :])
                bc = pool.tile([64, TQ], fp32, tag="bc", bufs=2,
                               name=f"bc_{j}_{pr}_{h}")
                nc.gpsimd.partition_broadcast(bc[:], den[:])
                rec = pool.tile([64, TQ], fp32, tag="rec", bufs=2,
                                name=f"rec_{j}_{pr}_{h}")
                nc.vector.reciprocal_approx_fast(rec[:], bc[:])
                nc.vector.tensor_mul(ob[(j, pr)][h * 64:(h + 1) * 64, :],
                                     ot[0:64, :], rec[:])

            # ---- main schedule ----
            # proj blocks 0,1 fully upfront; block j+2 paced across row j;
            # wproj rows 0..5 deferred and paced across rows 6..7.
            xts = {0: dma_x(0), 1: dma_x(1)}
            for tb in (0, 1):
                for u in make_units(tb, xts[tb]):
                    u()

            filler = []
            fill_emitted = [0]
            fill_tiles = 4 * 7 * 4 + 4 * 8 * 4  # attn tiles in rows 6+7
            fill_done = [0]

            for j in range(NTQ):
                if j + 2 < NTQ:
                    xts[j + 2] = dma_x(j + 2)
                    row_units = make_units(j + 2, xts[j + 2])
                else:
                    row_units = []
                n_units = len(row_units)
                row_tiles = 4 * (j + 1) * 4
                tcount = 0
                emitted = 0
                for pr in range(PAIRS):
                    ob[(j, pr)] = pool.tile(
                        [128, TQ], bf16,
                        tag=(f"ob{j}_{pr}" if j < 6 else "obx"),
                        bufs=(1 if j < 6 else 8),
                        name=f"ob_{j}_{pr}")
                    ot = [psum.tile([65, TQ], fp32, tag="ot", bufs=2,
                                    name=f"ot{h}_{pr}_{j}")
                          for h in range(2)]
                    for c in range(j + 1):
                        for (h, half) in ((0, 0), (1, 0), (0, 1), (1, 1)):
                            last = (c == j and h == 1 and half == 1)
                            post = None
                            if last:
                                def post(j=j, pr=pr, o0=ot[0], o1=ot[1]):
                                    normalize(j, pr, 0, o0)
                                    normalize(j, pr, 1, o1)
                            attn_tile(j, pr, c, h, half, ot[h], post)
                            tcount += 1
                            target = math.ceil(n_units * tcount / row_tiles)
                            while emitted < target:
                                row_units[emitted]()
                                emitted += 1
                            if j >= 6:
                                fill_done[0] += 1
                                ft = math.ceil(
                                    len(filler) * fill_done[0] / fill_tiles)
                                while fill_emitted[0] < ft:
                                    filler[fill_emitted[0]]()
                                    fill_emitted[0] += 1
                if j <= 5:
                    for mc in range(8):
                        filler.append(wproj_unit(j, mc))
                else:
                    flush_av()
                    for mc in range(8):
                        wproj_unit(j, mc)()
            flush_av()
            while fill_emitted[0] < len(filler):
                filler[fill_emitted[0]]()
                fill_emitted[0] += 1

    nc.compile()
    return nc


def _get_nc():
    if "nc" not in _CACHE:
        _CACHE["nc"] = _build_nc()
    return _CACHE["nc"]


def _in_maps(x, w_qkv, w_proj):
    import ml_dtypes
    bf16 = ml_dtypes.bfloat16
    p = np.arange(128, dtype=np.int32)
    msk = (p[:, None] <= p[None, :]).astype(bf16)
    maps = []
    for c in range(N_CORES):
        b, g = c // 2, c % 2
        wq = w_qkv[:, g * DL:(g + 1) * DL]
        wk = w_qkv[:, D + g * DL:D + (g + 1) * DL]
        wv = w_qkv[:, 2 * D + g * DL:2 * D + (g + 1) * DL]
        maps.append({
            "xt": np.ascontiguousarray(x[b].T).astype(bf16),
            "wqkv": np.ascontiguousarray(
                np.concatenate([wq, wk, wv], axis=1)).astype(bf16),
            "wp": np.ascontiguousarray(w_proj[g * DL:(g + 1) * DL, :]).astype(bf16),
            "msk": msk,
        })
    return maps


def _run(x, w_qkv, w_proj, trace=False):
    from concourse.bass_utils import run_bass_kernel_spmd

    nc = _get_nc()
    res = run_bass_kernel_spmd(nc, _in_maps(x, w_qkv, w_proj),
                               core_ids=list(range(N_CORES)), trace=trace)
    outs = [res.results[c]["yt"] for c in range(N_CORES)]
    y = np.stack([(outs[2 * b] + outs[2 * b + 1]).T for b in range(B)])
    return np.ascontiguousarray(y.astype(np.float32)), res


def kernel(x, w_qkv, w_proj):
    x = np.asarray(x, dtype=np.float32)
    w_qkv = np.asarray(w_qkv, dtype=np.float32)
    w_proj = np.asarray(w_proj, dtype=np.float32)
    y, _ = _run(x, w_qkv, w_proj, trace=False)
    return y


def kernel_traced(x, w_qkv, w_proj):
    """Test-only entry: run with NTFF profiling (needs the sibling prof_shim
    module; the graded kernel() path never imports it)."""
    import prof_shim
    prof_shim.install()
    y, res = _run(np.asarray(x, np.float32), np.asarray(w_qkv, np.float32),
                  np.asarray(w_proj, np.float32), trace=True)
    return y, res


# revision 13
# speedup vs baseline: 1.4112x; 1.0075x over previous
"""Multi-head causal attention (B=4, T=4096, D=1024, H=16) on 8 TRN2 NeuronCores.

Sharding: core c -> (batch b = c//2, head-group g = c%2 of 8 heads).
Host sums the two per-batch partials (w_proj row-split) and transposes.

v2 design (vs v1 baseline at 1233us):
  - bf16 operands everywhere (matmul rate unchanged vs fp32r, but halves
    SBUF/HBM footprint); fp32 PSUM accumulation throughout.
  - Everything SBUF-resident: K^T / V / Q^T live in SBUF between the QKV
    projection and attention -- no DRAM round-trip (v1 moved ~110MB).
  - No PE transposes: host supplies x^T, projection emits Q^T/K^T directly
    (W-stationary) and V in natural layout (x^T-stationary).
  - Causal diagonal blocks trimmed at 128-granularity: S / exp / mask / AV
    restricted to valid columns (-8% PE and ScalarE work).
  - Software pipeline: AV for tile k is emitted while tile k+1's S/exp are
    in flight, so the PE never sits on the just-issued exp. Projection and
    (deferred) output-projection units are paced into the attention stream
    as PE gap fillers; ScalarE (exp, ~580us) hides under PE (~670us).
  - softmax without max-subtraction (logits ~N(0,1)); denominator via a
    ones-column in V (comes out of the same AV matmul, PSUM row 64).
"""
import math

import numpy as np

B, T, D = 4, 4096, 1024
H, HD = 16, 64
N_CORES = 8
PAIRS = 4            # head-pairs per core (8 local heads)
DL = PAIRS * 128     # 512 = local q/k/v width
TQ = 512             # query block
NTQ = T // TQ        # 8

_CACHE = {}


def _build_nc():
    import concourse.tile as tile
    from concourse import bacc, mybir

    fp32 = mybir.dt.float32
    bf16 = mybir.dt.bfloat16
    AF = mybir.ActivationFunctionType

    nc = bacc.Bacc("TRN2", target_bir_lowering=False, debug=False,
                   num_devices=N_CORES)
    xt_d = nc.dram_tensor("xt", [D, T], bf16, kind="ExternalInput").ap()
    wqkv_d = nc.dram_tensor("wqkv", [D, 3 * DL], bf16, kind="ExternalInput").ap()
    wp_d = nc.dram_tensor("wp", [DL, D], bf16, kind="ExternalInput").ap()
    msk_d = nc.dram_tensor("msk", [128, 128], bf16, kind="ExternalInput").ap()
    yt_d = nc.dram_tensor("yt", [D, T], fp32, kind="ExternalOutput").ap()

    with tile.TileContext(nc) as tc:
        with (
            tc.tile_pool(name="sb", bufs=1) as pool,
            tc.tile_pool(name="ps", bufs=1, space="PSUM") as psum,
        ):
            # wqkv split per output-column group so the first projection
            # units only wait on their own 256KB slice, not the full 3MB.
            wqkv = pool.tile([128, 8, 3 * DL], bf16, tag="wqkv")

            def dma_wqkv(lo, hi):
                nc.sync.dma_start(
                    wqkv[:, :, lo:hi],
                    wqkv_d[:, lo:hi].rearrange("(a p) f -> p a f", p=128))

            wp = pool.tile([128, 4, D], bf16, tag="wp")
            msk = pool.tile([128, 128], bf16, tag="msk")

            # K^T and V resident for all 8 token-blocks; Q^T rotates (only
            # row j reads q block j; block j+2 is written during row j).
            kT = [pool.tile([128, PAIRS, TQ], bf16, tag=f"kT{t}",
                            name=f"kT_{t}")
                  for t in range(NTQ)]
            vb = [pool.tile([128, PAIRS, 4, 2, 65], bf16, tag=f"vb{t}",
                            name=f"vb_{t}")
                  for t in range(NTQ)]
            for t in range(NTQ):
                # ones column per head -> softmax denominator out of AV matmul
                nc.vector.memset(vb[t][:, :, :, :, 64:65], 1.0)

            qT = {}

            def dma_x(tb, split=False):
                xt = pool.tile([128, 8, TQ], bf16, tag="xt", bufs=2,
                               name=f"xt_{tb}")
                src = xt_d[:, tb * TQ:(tb + 1) * TQ]
                if split:  # halves so the first matmuls start sooner
                    nc.sync.dma_start(
                        xt[:, 0:4, :],
                        src[0:512, :].rearrange("(a p) t -> p a t", p=128))
                    nc.sync.dma_start(
                        xt[:, 4:8, :],
                        src[512:1024, :].rearrange("(a p) t -> p a t", p=128))
                else:
                    nc.sync.dma_start(
                        xt[:], src.rearrange("(a p) t -> p a t", p=128))
                return xt

            def copy_engine():
                # GPSIMD/Pool cannot read PSUM on TRN2; DVE does all
                # PSUM->SBUF drains.
                return nc.vector

            def make_units(tb, xt):
                """12 projection PE-work units for token rows [tb*512,+512)."""
                units = []

                def qk_unit(fc):
                    def run():
                        if fc == 0:
                            qT[tb] = pool.tile([128, PAIRS, TQ], bf16,
                                               tag="qt", bufs=3,
                                               name=f"qt_{tb}")
                        pp = psum.tile([128, TQ], fp32, tag="pp", bufs=2,
                                       name=f"pqk_{tb}_{fc}")
                        for kb in range(8):
                            nc.tensor.matmul(
                                pp[:], wqkv[:, kb, fc * 128:(fc + 1) * 128],
                                xt[:, kb, :], start=(kb == 0), stop=(kb == 7))
                        dst = qT[tb] if fc < 4 else kT[tb]
                        copy_engine().tensor_copy(dst[:, fc % 4, :], pp[:])
                    return run

                def v_unit(s):
                    def run():
                        pp = psum.tile([128, TQ], fp32, tag="pp", bufs=2,
                                       name=f"pv_{tb}_{s}")
                        for kb in range(8):
                            nc.tensor.matmul(
                                pp[:], xt[:, kb, s * 128:(s + 1) * 128],
                                wqkv[:, kb, 2 * DL:3 * DL],
                                start=(kb == 0), stop=(kb == 7))
                        copy_engine().tensor_copy(
                            vb[tb][:, :, s, :, 0:64],
                            pp[:].rearrange("p (a h e) -> p a h e", a=4, h=2))
                    return run

                for fc in range(8):
                    units.append(qk_unit(fc))
                for s in range(4):
                    units.append(v_unit(s))
                return units

            ob = {}  # (j, pr) -> attention-output SBUF tile [128, TQ] bf16

            def wproj_pair(j, mc0):
                """Output-proj for mc0, mc0+1, kc-major: the kc=3 matmuls
                (gated on the last pair's normalize) come last, so the PE
                isn't stalled mid-unit waiting for ob tiles."""
                def run():
                    yps = [psum.tile([128, TQ], fp32, tag="pp", bufs=2,
                                     name=f"yp_{j}_{mc0 + i}")
                           for i in range(2)]
                    for kc in range(PAIRS):
                        for i in range(2):
                            mc = mc0 + i
                            nc.tensor.matmul(
                                yps[i][:], wp[:, kc, mc * 128:(mc + 1) * 128],
                                ob[(j, kc)][:],
                                start=(kc == 0), stop=(kc == PAIRS - 1))
                    for i in range(2):
                        mc = mc0 + i
                        ys = pool.tile([128, TQ], fp32, tag="ys", bufs=2,
                                       name=f"ys_{j}_{mc}")
                        nc.vector.tensor_copy(ys[:], yps[i][:])
                        nc.sync.dma_start(
                            yt_d[mc * 128:(mc + 1) * 128, j * TQ:(j + 1) * TQ],
                            ys[:])
                return run

            # ---- attention tile pipeline (AV delayed by one tile) ----
            pending = [None]

            def do_av(j, pr, c, h, half, ot, ex):
                for tkb in range(2):
                    blk = half * 2 + tkb
                    d = 128 * blk if c == j else 0
                    nc.tensor.matmul(
                        ot[:, d:TQ], vb[c][:, pr, blk, h, :],
                        ex[:, tkb, d:TQ],
                        start=(c == 0 and half == 0 and tkb == 0),
                        stop=(c == j and half == 1 and tkb == 1))

            def flush_av():
                if pending[0] is None:
                    return
                args, post = pending[0]
                pending[0] = None
                do_av(*args)
                if post is not None:
                    post()

            def attn_tile(j, pr, c, h, half, ot, post=None):
                st = psum.tile([128, 2, TQ], fp32, tag="st", bufs=2,
                               name=f"st_{j}_{pr}_{c}_{h}_{half}")
                for tkb in range(2):
                    blk = half * 2 + tkb
                    d = 128 * blk if c == j else 0
                    nc.tensor.matmul(
                        st[:, tkb, d:TQ],
                        kT[c][h * 64:(h + 1) * 64, pr, blk * 128:(blk + 1) * 128],
                        qT[j][h * 64:(h + 1) * 64, pr, d:TQ],
                        start=True, stop=True)
                ex = pool.tile([128, 2, TQ], bf16, tag="ex", bufs=4,
                               name=f"ex_{j}_{pr}_{c}_{h}_{half}")
                if c < j:
                    nc.scalar.activation(ex[:], st[:], AF.Exp, scale=0.125)
                else:
                    for tkb in range(2):
                        d = 128 * (half * 2 + tkb)
                        nc.scalar.activation(ex[:, tkb, d:TQ], st[:, tkb, d:TQ],
                                             AF.Exp, scale=0.125)
                        nc.vector.tensor_mul(ex[:, tkb, d:d + 128],
                                             ex[:, tkb, d:d + 128], msk[:])
                flush_av()
                pending[0] = ((j, pr, c, h, half, ot, ex), post)

            def normalize_pair(j, pr, ots):
                """h0/h1 chains interleaved across DVE and Pool."""
                den = [pool.tile([1, TQ], fp32, tag="den", bufs=2,
                                 name=f"den_{j}_{pr}_{h}") for h in range(2)]
                bc = [pool.tile([64, TQ], fp32, tag="bc", bufs=2,
                                name=f"bc_{j}_{pr}_{h}") for h in range(2)]
                rec = [pool.tile([64, TQ], fp32, tag="rec", bufs=2,
                                 name=f"rec_{j}_{pr}_{h}") for h in range(2)]
                for h in range(2):
                    nc.vector.tensor_copy(den[h][:], ots[h][64:65, :])
                for h in range(2):
                    nc.gpsimd.partition_broadcast(bc[h][:], den[h][:])
                for h in range(2):
                    nc.vector.reciprocal_approx_fast(rec[h][:], bc[h][:])
                for h in range(2):
                    nc.vector.tensor_mul(ob[(j, pr)][h * 64:(h + 1) * 64, :],
                                         ots[h][0:64, :], rec[h][:])

            # ---- main schedule ----
            # proj blocks 0,1 fully upfront; block j+2 paced across row j;
            # wproj rows 0..5 deferred and paced across rows 6..7.
            # DMA order: first units' operands first.
            dma_wqkv(0, 128)
            xts = {0: dma_x(0, split=True)}
            dma_wqkv(128, 512)
            dma_wqkv(512, 1024)
            dma_wqkv(1024, 3 * DL)
            nc.sync.dma_start(msk[:], msk_d[:])
            xts[1] = dma_x(1, split=True)
            nc.sync.dma_start(wp[:], wp_d.rearrange("(a p) f -> p a f", p=128))
            for tb in (0, 1):
                for u in make_units(tb, xts[tb]):
                    u()

            filler = []
            fill_emitted = [0]
            fill_tiles = 4 * 7 * 4 + 4 * 8 * 4  # attn tiles in rows 6+7
            fill_done = [0]

            for j in range(NTQ):
                if j + 2 < NTQ:
                    xts[j + 2] = dma_x(j + 2)
                    row_units = make_units(j + 2, xts[j + 2])
                else:
                    row_units = []
                n_units = len(row_units)
                row_tiles = 4 * (j + 1) * 4
                tcount = 0
                emitted = 0
                for pr in range(PAIRS):
                    ob[(j, pr)] = pool.tile(
                        [128, TQ], bf16,
                        tag=(f"ob{j}_{pr}" if j < 6 else "obx"),
                        bufs=(1 if j < 6 else 8),
                        name=f"ob_{j}_{pr}")
                    ot = [psum.tile([65, TQ], fp32, tag="ot", bufs=2,
                                    name=f"ot{h}_{pr}_{j}")
                          for h in range(2)]
                    for c in range(j + 1):
                        for (h, half) in ((0, 0), (1, 0), (0, 1), (1, 1)):
                            last = (c == j and h == 1 and half == 1)
                            post = None
                            if last:
                                def post(j=j, pr=pr, ots=tuple(ot)):
                                    normalize_pair(j, pr, ots)
                            attn_tile(j, pr, c, h, half, ot[h], post)
                            tcount += 1
                            target = math.ceil(n_units * tcount / row_tiles)
                            while emitted < target:
                                row_units[emitted]()
                                emitted += 1
                            if j >= 6:
                                fill_done[0] += 1
                                ft = math.ceil(
                                    len(filler) * fill_done[0] / fill_tiles)
                                while fill_emitted[0] < ft:
                                    filler[fill_emitted[0]]()
                                    fill_emitted[0] += 1
                if j <= 5:
                    for mc0 in range(0, 8, 2):
                        filler.append(wproj_pair(j, mc0))
                else:
                    flush_av()
                    for mc0 in range(0, 8, 2):
                        wproj_pair(j, mc0)()
            flush_av()
            while fill_emitted[0] < len(filler):
                filler[fill_emitted[0]]()
                fill_emitted[0] += 1

    nc.compile()
    return nc


def _get_nc():
    if "nc" not in _CACHE:
        _CACHE["nc"] = _build_nc()
    return _CACHE["nc"]


def _in_maps(x, w_qkv, w_proj):
    import ml_dtypes
    bf16 = ml_dtypes.bfloat16
    p = np.arange(128, dtype=np.int32)
    msk = (p[:, None] <= p[None, :]).astype(bf16)
    maps = []
    for c in range(N_CORES):
        b, g = c // 2, c % 2
        wq = w_qkv[:, g * DL:(g + 1) * DL]
        wk = w_qkv[:, D + g * DL:D + (g + 1) * DL]
        wv = w_qkv[:, 2 * D + g * DL:2 * D + (g + 1) * DL]
        maps.append({
            "xt": np.ascontiguousarray(x[b].T).astype(bf16),
            "wqkv": np.ascontiguousarray(
                np.concatenate([wq, wk, wv], axis=1)).astype(bf16),
            "wp": np.ascontiguousarray(w_proj[g * DL:(g + 1) * DL, :]).astype(bf16),
            "msk": msk,
        })
    return maps


def _run(x, w_qkv, w_proj, trace=False):
    from concourse.bass_utils import run_bass_kernel_spmd

    nc = _get_nc()
    res = run_bass_kernel_spmd(nc, _in_maps(x, w_qkv, w_proj),
                               core_ids=list(range(N_CORES)), trace=trace)
    outs = [res.results[c]["yt"] for c in range(N_CORES)]
    y = np.stack([(outs[2 * b] + outs[2 * b + 1]).T for b in range(B)])
    return np.ascontiguousarray(y.astype(np.float32)), res


def kernel(x, w_qkv, w_proj):
    x = np.asarray(x, dtype=np.float32)
    w_qkv = np.asarray(w_qkv, dtype=np.float32)
    w_proj = np.asarray(w_proj, dtype=np.float32)
    y, _ = _run(x, w_qkv, w_proj, trace=False)
    return y


def kernel_traced(x, w_qkv, w_proj):
    """Test-only entry: run with NTFF profiling (needs the sibling prof_shim
    module; the graded kernel() path never imports it)."""
    import prof_shim
    prof_shim.install()
    y, res = _run(np.asarray(x, np.float32), np.asarray(w_qkv, np.float32),
                  np.asarray(w_proj, np.float32), trace=True)
    return y, res


# revision 19
# speedup vs baseline: 1.4161x; 1.0035x over previous
"""Multi-head causal attention (B=4, T=4096, D=1024, H=16) on 8 TRN2 NeuronCores.

Sharding: core c -> (batch b = c//2, head-group g = c%2 of 8 heads).
Host sums the two per-batch partials (w_proj row-split) and transposes.

v2 design (vs v1 baseline at 1233us):
  - bf16 operands everywhere (matmul rate unchanged vs fp32r, but halves
    SBUF/HBM footprint); fp32 PSUM accumulation throughout.
  - Everything SBUF-resident: K^T / V / Q^T live in SBUF between the QKV
    projection and attention -- no DRAM round-trip (v1 moved ~110MB).
  - No PE transposes: host supplies x^T, projection emits Q^T/K^T directly
    (W-stationary) and V in natural layout (x^T-stationary).
  - Causal diagonal blocks trimmed at 128-granularity: S / exp / mask / AV
    restricted to valid columns (-8% PE and ScalarE work).
  - Software pipeline: AV for tile k is emitted while tile k+1's S/exp are
    in flight, so the PE never sits on the just-issued exp. Projection and
    (deferred) output-projection units are paced into the attention stream
    as PE gap fillers; ScalarE (exp, ~580us) hides under PE (~670us).
  - softmax without max-subtraction (logits ~N(0,1)); denominator via a
    ones-column in V (comes out of the same AV matmul, PSUM row 64).
"""
import math

import numpy as np

B, T, D = 4, 4096, 1024
H, HD = 16, 64
N_CORES = 8
PAIRS = 4            # head-pairs per core (8 local heads)
DL = PAIRS * 128     # 512 = local q/k/v width
TQ = 512             # query block
NTQ = T // TQ        # 8

_CACHE = {}


def _build_nc():
    import concourse.tile as tile
    from concourse import bacc, mybir

    fp32 = mybir.dt.float32
    bf16 = mybir.dt.bfloat16
    AF = mybir.ActivationFunctionType

    nc = bacc.Bacc("TRN2", target_bir_lowering=False, debug=False,
                   num_devices=N_CORES)
    xt_d = nc.dram_tensor("xt", [D, T], bf16, kind="ExternalInput").ap()
    wqkv_d = nc.dram_tensor("wqkv", [D, 3 * DL], bf16, kind="ExternalInput").ap()
    wp_d = nc.dram_tensor("wp", [DL, D], bf16, kind="ExternalInput").ap()
    msk_d = nc.dram_tensor("msk", [128, 128], bf16, kind="ExternalInput").ap()
    yt_d = nc.dram_tensor("yt", [D, T], fp32, kind="ExternalOutput").ap()

    with tile.TileContext(nc) as tc:
        with (
            tc.tile_pool(name="sb", bufs=1) as pool,
            tc.tile_pool(name="ps", bufs=1, space="PSUM") as psum,
        ):
            # wqkv split per output-column group so the first projection
            # units only wait on their own 256KB slice, not the full 3MB.
            wqkv = pool.tile([128, 8, 3 * DL], bf16, tag="wqkv")

            def dma_wqkv(lo, hi):
                nc.sync.dma_start(
                    wqkv[:, :, lo:hi],
                    wqkv_d[:, lo:hi].rearrange("(a p) f -> p a f", p=128))

            wp = pool.tile([128, 4, D], bf16, tag="wp")
            msk = pool.tile([128, 128], bf16, tag="msk")

            # K^T and V resident for all 8 token-blocks; Q^T rotates (only
            # row j reads q block j; block j+2 is written during row j).
            kT = [pool.tile([128, PAIRS, TQ], bf16, tag=f"kT{t}",
                            name=f"kT_{t}")
                  for t in range(NTQ)]
            vb = [pool.tile([128, PAIRS, 4, 2, 65], bf16, tag=f"vb{t}",
                            name=f"vb_{t}")
                  for t in range(NTQ)]
            for t in range(NTQ):
                # ones column per head -> softmax denominator out of AV matmul
                nc.vector.memset(vb[t][:, :, :, :, 64:65], 1.0)

            qT = {}

            def dma_x(tb, split=False):
                xt = pool.tile([128, 8, TQ], bf16, tag="xt", bufs=2,
                               name=f"xt_{tb}")
                src = xt_d[:, tb * TQ:(tb + 1) * TQ]
                if split:  # halves so the first matmuls start sooner
                    nc.sync.dma_start(
                        xt[:, 0:4, :],
                        src[0:512, :].rearrange("(a p) t -> p a t", p=128))
                    nc.sync.dma_start(
                        xt[:, 4:8, :],
                        src[512:1024, :].rearrange("(a p) t -> p a t", p=128))
                else:
                    nc.sync.dma_start(
                        xt[:], src.rearrange("(a p) t -> p a t", p=128))
                return xt

            def copy_engine():
                # GPSIMD/Pool cannot read PSUM on TRN2; DVE does all
                # PSUM->SBUF drains.
                return nc.vector

            def make_units(tb, xt):
                """12 projection PE-work units for token rows [tb*512,+512)."""
                units = []

                def qk_unit(fc):
                    def run():
                        if fc == 0:
                            qT[tb] = pool.tile([128, PAIRS, TQ], bf16,
                                               tag="qt", bufs=3,
                                               name=f"qt_{tb}")
                        pp = psum.tile([128, TQ], fp32, tag="pp", bufs=2,
                                       name=f"pqk_{tb}_{fc}")
                        for kb in range(8):
                            nc.tensor.matmul(
                                pp[:], wqkv[:, kb, fc * 128:(fc + 1) * 128],
                                xt[:, kb, :], start=(kb == 0), stop=(kb == 7))
                        dst = qT[tb] if fc < 4 else kT[tb]
                        copy_engine().tensor_copy(dst[:, fc % 4, :], pp[:])
                    return run

                def v_unit(s):
                    def run():
                        pp = psum.tile([128, TQ], fp32, tag="pp", bufs=2,
                                       name=f"pv_{tb}_{s}")
                        for kb in range(8):
                            nc.tensor.matmul(
                                pp[:], xt[:, kb, s * 128:(s + 1) * 128],
                                wqkv[:, kb, 2 * DL:3 * DL],
                                start=(kb == 0), stop=(kb == 7))
                        copy_engine().tensor_copy(
                            vb[tb][:, :, s, :, 0:64],
                            pp[:].rearrange("p (a h e) -> p a h e", a=4, h=2))
                    return run

                for fc in range(8):
                    units.append(qk_unit(fc))
                for s in range(4):
                    units.append(v_unit(s))
                return units

            ob = {}  # (j, pr) -> attention-output SBUF tile [128, TQ] bf16

            def wproj_pair(j, mc0):
                """Output-proj for mc0, mc0+1, kc-major: the kc=3 matmuls
                (gated on the last pair's normalize) come last, so the PE
                isn't stalled mid-unit waiting for ob tiles."""
                def run():
                    yps = [psum.tile([128, TQ], fp32, tag="pp", bufs=2,
                                     name=f"yp_{j}_{mc0 + i}")
                           for i in range(2)]
                    for kc in range(PAIRS):
                        for i in range(2):
                            mc = mc0 + i
                            nc.tensor.matmul(
                                yps[i][:], wp[:, kc, mc * 128:(mc + 1) * 128],
                                ob[(j, kc)][:],
                                start=(kc == 0), stop=(kc == PAIRS - 1))
                    for i in range(2):
                        mc = mc0 + i
                        ys = pool.tile([128, TQ], fp32, tag="ys", bufs=2,
                                       name=f"ys_{j}_{mc}")
                        nc.vector.tensor_copy(ys[:], yps[i][:])
                        nc.sync.dma_start(
                            yt_d[mc * 128:(mc + 1) * 128, j * TQ:(j + 1) * TQ],
                            ys[:])
                return run

            # ---- attention tile pipeline (AV delayed by one tile) ----
            pending = [None]

            def do_av(j, pr, c, h, half, ot, ex):
                for tkb in range(2):
                    blk = half * 2 + tkb
                    d = 128 * blk if c == j else 0
                    nc.tensor.matmul(
                        ot[:, d:TQ], vb[c][:, pr, blk, h, :],
                        ex[:, tkb, d:TQ],
                        start=(c == 0 and half == 0 and tkb == 0),
                        stop=(c == j and half == 1 and tkb == 1))

            def flush_av():
                if pending[0] is None:
                    return
                args, post = pending[0]
                pending[0] = None
                do_av(*args)
                if post is not None:
                    post()

            def attn_tile(j, pr, c, h, half, ot, post=None):
                st = psum.tile([128, 2, TQ], fp32, tag="st", bufs=2,
                               name=f"st_{j}_{pr}_{c}_{h}_{half}")
                for tkb in range(2):
                    blk = half * 2 + tkb
                    d = 128 * blk if c == j else 0
                    nc.tensor.matmul(
                        st[:, tkb, d:TQ],
                        kT[c][h * 64:(h + 1) * 64, pr, blk * 128:(blk + 1) * 128],
                        qT[j][h * 64:(h + 1) * 64, pr, d:TQ],
                        start=True, stop=True)
                ex = pool.tile([128, 2, TQ], bf16, tag="ex", bufs=4,
                               name=f"ex_{j}_{pr}_{c}_{h}_{half}")
                if c < j:
                    nc.scalar.activation(ex[:], st[:], AF.Exp, scale=0.125)
                else:
                    for tkb in range(2):
                        d = 128 * (half * 2 + tkb)
                        nc.scalar.activation(ex[:, tkb, d:TQ], st[:, tkb, d:TQ],
                                             AF.Exp, scale=0.125)
                        nc.vector.tensor_mul(ex[:, tkb, d:d + 128],
                                             ex[:, tkb, d:d + 128], msk[:])
                flush_av()
                pending[0] = ((j, pr, c, h, half, ot, ex), post)

            def normalize_pair(j, pr, ots):
                """h0/h1 chains interleaved across DVE and Pool."""
                den = [pool.tile([1, TQ], fp32, tag="den", bufs=2,
                                 name=f"den_{j}_{pr}_{h}") for h in range(2)]
                bc = [pool.tile([64, TQ], fp32, tag="bc", bufs=2,
                                name=f"bc_{j}_{pr}_{h}") for h in range(2)]
                rec = [pool.tile([64, TQ], fp32, tag="rec", bufs=2,
                                 name=f"rec_{j}_{pr}_{h}") for h in range(2)]
                for h in range(2):
                    nc.vector.tensor_copy(den[h][:], ots[h][64:65, :])
                for h in range(2):
                    nc.gpsimd.partition_broadcast(bc[h][:], den[h][:])
                for h in range(2):
                    nc.vector.reciprocal_approx_fast(rec[h][:], bc[h][:])
                for h in range(2):
                    nc.vector.tensor_mul(ob[(j, pr)][h * 64:(h + 1) * 64, :],
                                         ots[h][0:64, :], rec[h][:])

            # ---- main schedule ----
            # proj blocks 0,1 fully upfront; block j+2 paced across row j;
            # wproj rows 0..5 deferred and paced across rows 6..7.
            # DMA order: first units' operands first.
            dma_wqkv(0, 128)
            xts = {0: dma_x(0, split=True)}
            dma_wqkv(128, 512)
            dma_wqkv(512, 1024)
            dma_wqkv(1024, 3 * DL)
            nc.sync.dma_start(msk[:], msk_d[:])
            xts[1] = dma_x(1, split=True)
            nc.sync.dma_start(wp[:], wp_d.rearrange("(a p) f -> p a f", p=128))
            for tb in (0, 1):
                for u in make_units(tb, xts[tb]):
                    u()

            filler = []
            fill_emitted = [0]
            fill_tiles = 4 * 7 * 4 + 4 * 8 * 4  # attn tiles in rows 6+7
            fill_done = [0]

            for j in range(NTQ):
                if j + 2 < NTQ:
                    xts[j + 2] = dma_x(j + 2)
                    row_units = make_units(j + 2, xts[j + 2])
                else:
                    row_units = []
                n_units = len(row_units)
                row_tiles = 4 * (j + 1) * 4
                tcount = 0
                emitted = 0
                for pr in range(PAIRS):
                    ob[(j, pr)] = pool.tile(
                        [128, TQ], bf16,
                        tag=(f"ob{j}_{pr}" if j < 6 else "obx"),
                        bufs=(1 if j < 6 else 8),
                        name=f"ob_{j}_{pr}")
                    ot = [psum.tile([65, TQ], fp32, tag="ot", bufs=2,
                                    name=f"ot{h}_{pr}_{j}")
                          for h in range(2)]
                    for c in range(j + 1):
                        for (h, half) in ((0, 0), (1, 0), (0, 1), (1, 1)):
                            last = (c == j and h == 1 and half == 1)
                            post = None
                            if last:
                                def post(j=j, pr=pr, ots=tuple(ot)):
                                    normalize_pair(j, pr, ots)
                            attn_tile(j, pr, c, h, half, ot[h], post)
                            tcount += 1
                            target = math.ceil(n_units * tcount / row_tiles)
                            while emitted < target:
                                row_units[emitted]()
                                emitted += 1
                            if j >= 6:
                                fill_done[0] += 1
                                ft = math.ceil(
                                    len(filler) * fill_done[0] / fill_tiles)
                                while fill_emitted[0] < ft:
                                    filler[fill_emitted[0]]()
                                    fill_emitted[0] += 1
                if j <= 5:
                    for mc0 in range(0, 8, 2):
                        filler.append(wproj_pair(j, mc0))
                else:
                    flush_av()
                    for mc0 in range(0, 8, 2):
                        wproj_pair(j, mc0)()
            flush_av()
            while fill_emitted[0] < len(filler):
                filler[fill_emitted[0]]()
                fill_emitted[0] += 1

    nc.compile()
    return nc


def _get_nc():
    if "nc" not in _CACHE:
        _CACHE["nc"] = _build_nc()
    return _CACHE["nc"]


def _in_maps(x, w_qkv, w_proj):
    import ml_dtypes
    bf16 = ml_dtypes.bfloat16
    p = np.arange(128, dtype=np.int32)
    msk = (p[:, None] <= p[None, :]).astype(bf16)
    maps = []
    for c in range(N_CORES):
        b, g = c // 2, c % 2
        wq = w_qkv[:, g * DL:(g + 1) * DL]
        wk = w_qkv[:, D + g * DL:D + (g + 1) * DL]
        wv = w_qkv[:, 2 * D + g * DL:2 * D + (g + 1) * DL]
        maps.append({
            "xt": np.ascontiguousarray(x[b].T).astype(bf16),
            "wqkv": np.ascontiguousarray(
                np.concatenate([wq, wk, wv], axis=1)).astype(bf16),
            "wp": np.ascontiguousarray(w_proj[g * DL:(g + 1) * DL, :]).astype(bf16),
            "msk": msk,
        })
    return maps


def _run(x, w_qkv, w_proj, trace=False):
    from concourse.bass_utils import run_bass_kernel_spmd

    nc = _get_nc()
    res = run_bass_kernel_spmd(nc, _in_maps(x, w_qkv, w_proj),
                               core_ids=list(range(N_CORES)), trace=trace)
    outs = [res.results[c]["yt"] for c in range(N_CORES)]
    y = np.stack([(outs[2 * b] + outs[2 * b + 1]).T for b in range(B)])
    return np.ascontiguousarray(y.astype(np.float32)), res


def kernel(x, w_qkv, w_proj):
    x = np.asarray(x, dtype=np.float32)
    w_qkv = np.asarray(w_qkv, dtype=np.float32)
    w_proj = np.asarray(w_proj, dtype=np.float32)
    y, _ = _run(x, w_qkv, w_proj, trace=False)
    return y


def kernel_traced(x, w_qkv, w_proj):
    """Test-only entry: run with NTFF profiling (needs the sibling prof_shim
    module; the graded kernel() path never imports it)."""
    import prof_shim
    prof_shim.install()
    y, res = _run(np.asarray(x, np.float32), np.asarray(w_qkv, np.float32),
                  np.asarray(w_proj, np.float32), trace=True)
    return y, res
